# revision 24
# baseline (speedup 1.0000x reference)
"""BiRNN (Bowman SNLI) Trainium2 kernel.

Sharding: 8 cores = 4 LSTM directions x 2 batch halves (SPMD — same program,
per-core weights/inputs differ). Each core runs one LSTM (batch 128, T=128)
in "natural" layout: per step z = [x_t, h, 1] @ W_aug accumulated in PSUM with
stationary = x_t^T / h^T chunks and moving = weight columns (N=512, float32r).
Gates are column-reordered [i, f, o, j] host-side (so one sigmoid covers
i|f|o) and the forget bias is folded into b. h^T for the next step comes from
4 PE transposes. Final cell states are AllGathered within each batch-half
group of 4 cores, then every core runs the small MLP in transposed layout
(bf16) and writes logits^T [3, 128]; the host reads cores 0 and 4.
"""
import numpy as np
import ml_dtypes

# Harness-visible constants
B, T, E, H, F = 256, 128, 300, 512, 1024
BC = 128          # batch per core
N_CORES = 8
EP = 384          # padded x feature dim (300 x + 1 bias + pad)
KX = 3            # x stationary chunks (last has 45 valid rows)
KH = 4            # h stationary chunks

_cache = {}


def _apply_tile_patch():
    """walrus here allows ONE semaphore wait per instruction; Tile's tail
    drain (and occasionally other instructions) get more. Split extra waits
    onto same-engine NoOp carriers inserted immediately before."""
    import concourse.tile as tile
    import concourse.mybir as mybir
    from concourse.tile import ScopedClock

    if getattr(tile.TileContext, "_multiwait_patched", False):
        return

    def split_multiwait(nc):
        for f in nc.m.functions:
            for bb in f.blocks:
                insts = bb.instructions
                if not any(
                    i.sync_info is not None and len(i.sync_info.on_wait) > 1
                    for i in insts
                ):
                    continue
                new = []
                for inst in insts:
                    si = inst.sync_info
                    if si is not None and len(si.on_wait) > 1:
                        waits = list(si.on_wait)
                        for w in waits[:-1]:
                            carrier = mybir.InstNoOp(
                                name=nc.get_next_instruction_name(), ins=[], outs=[]
                            )
                            carrier.engine = inst.engine
                            carrier.sync_info = mybir.SyncInfo(
                                on_wait=[w], on_update=[]
                            )
                            nc.register_instruction(carrier, overwrite=True)
                            new.append(carrier)
                        si.on_wait = [waits[-1]]
                    new.append(inst)
                bb.instructions = new

    def _patched_drain_and_barrier(self, tick_clock, wait_clock):
        nc = self.nc
        drain_inst = nc.sync.drain()
        wait_clock.add_sem_waits(
            drain_inst.ins, ScopedClock({None: tick_clock.global_clock})
        )
        nc.all_engine_barrier()
        assert self.sems is not None
        popped = nc._tile_sem_poison_stack.pop()
        assert popped is self._sem_poison
        nc.clear_and_free_semaphores(list(self.sems.allocated().values()))
        nc.all_engine_barrier()
        split_multiwait(nc)

    tile.TileContext._drain_and_barrier = _patched_drain_and_barrier
    tile.TileContext._multiwait_patched = True


def _build_nc(t_steps=T, ldt="f32r"):
    _apply_tile_patch()
    from contextlib import ExitStack
    import concourse.bass as bass
    import concourse.tile as tile
    from concourse import mybir

    f32 = mybir.dt.float32
    f32r = mybir.dt.float32r
    bf16 = mybir.dt.bfloat16
    ldtype = {"f32r": f32r, "bf16": bf16}[ldt]
    AF = mybir.ActivationFunctionType

    nc = bass.Bass("TRN2", target_bir_lowering=False, debug=False,
                   num_devices=N_CORES)

    zx_d = nc.dram_tensor("zx", [t_steps, 128, 4 * H], ldtype, kind="ExternalInput").ap()
    wl_d = nc.dram_tensor("wl", [128, KH, 4 * H], ldtype, kind="ExternalInput").ap()
    w1_d = nc.dram_tensor("w1", [128, 16, F], bf16, kind="ExternalInput").ap()
    w2_d = nc.dram_tensor("w2", [128, 8, F], bf16, kind="ExternalInput").ap()
    w3_d = nc.dram_tensor("w3", [128, 8, F], bf16, kind="ExternalInput").ap()
    w4_d = nc.dram_tensor("w4", [128, 8, 3], bf16, kind="ExternalInput").ap()
    b1_d = nc.dram_tensor("b1", [1, F], bf16, kind="ExternalInput").ap()
    b2_d = nc.dram_tensor("b2", [1, F], bf16, kind="ExternalInput").ap()
    b3_d = nc.dram_tensor("b3", [1, F], bf16, kind="ExternalInput").ap()
    b4_d = nc.dram_tensor("b4", [1, 3], bf16, kind="ExternalInput").ap()
    ones_d = nc.dram_tensor("ones", [1, 128], bf16, kind="ExternalInput").ap()
    idr_d = nc.dram_tensor("identr", [128, 128], ldtype, kind="ExternalInput").ap()
    idb_d = nc.dram_tensor("identb", [128, 128], bf16, kind="ExternalInput").ap()
    out_d = nc.dram_tensor("logitsT", [3, 128], f32, kind="ExternalOutput").ap()

    cgin = nc.dram_tensor("cgin", [128, H], bf16)
    cgout = nc.dram_tensor("cgout", [4, 128, H], bf16)

    G4 = 4 * H  # 2048

    with tile.TileContext(nc) as tc, ExitStack() as ctx:
        wp = ctx.enter_context(tc.tile_pool(name="weights", bufs=1))
        wl_sb = wp.tile([128, KH, G4], ldtype, tag="wl")
        w1_sb = wp.tile([128, 16, F], bf16, tag="w1")
        w2_sb = wp.tile([128, 8, F], bf16, tag="w2")
        w3_sb = wp.tile([128, 8, F], bf16, tag="w3")
        w4_sb = wp.tile([128, 8, 3], bf16, tag="w4")
        b1_sb = wp.tile([1, F], bf16, tag="b1")
        b2_sb = wp.tile([1, F], bf16, tag="b2")
        b3_sb = wp.tile([1, F], bf16, tag="b3")
        b4_sb = wp.tile([1, 3], bf16, tag="b4")
        ones_sb = wp.tile([1, 128], bf16, tag="ones")
        idr_sb = wp.tile([128, 128], ldtype, tag="idr")
        idb_sb = wp.tile([128, 128], bf16, tag="idb")
        # LSTM-critical loads first (identity gates the first inject MMs;
        # wl chunk k gates step-1 h-MMs for that k)
        nc.sync.dma_start(idr_sb[:], idr_d[:])
        for k in range(KH):
            nc.sync.dma_start(wl_sb[:, k, :], wl_d[:, k])
        # MLP weights aren't needed until after the T loop — emitted late
        # (below) so their DMA doesn't delay the LSTM start.
        _mlp_loads = [(w1_sb, w1_d), (w2_sb, w2_d), (w3_sb, w3_d),
                      (w4_sb, w4_d), (b1_sb, b1_d), (b2_sb, b2_d),
                      (b3_sb, b3_d), (b4_sb, b4_d), (ones_sb, ones_d),
                      (idb_sb, idb_d)]

        xp = ctx.enter_context(tc.tile_pool(name="xsteps", bufs=4))
        sp = ctx.enter_context(tc.tile_pool(name="state", bufs=2))

        c_prev = None
        hT_prev = None
        cT_bf = None

        # gate layout [i | f | o | j]; bank order chosen so the gates the
        # elementwise tail needs first (i, j) finish first
        BANK_ORDER = (0, 3, 1, 2)
        GATE_FUNC = {0: AF.Sigmoid, 1: AF.Sigmoid, 2: AF.Sigmoid, 3: AF.Tanh}

        with tc.tile_pool(name="zpsum", bufs=1, space="PSUM") as zpool, \
             tc.tile_pool(name="trpsum", bufs=2, space="PSUM") as trpool, \
             tc.tile_pool(name="trbpsum", bufs=1, space="PSUM") as trbpool, \
             tc.tile_pool(name="scrpsum", bufs=1, space="PSUM") as scrpool:
            for t in range(t_steps):
                if t == 2:
                    for sb_t, d in _mlp_loads:
                        nc.sync.dma_start(sb_t[:], d[:])
                zx_sb = xp.tile([128, G4], ldtype, tag="zx")
                nc.sync.dma_start(zx_sb[:], zx_d[t])

                # one PSUM tile per gate bank so the next step's inject-MMs
                # only wait on that bank's ACT read, not all four
                zb = [
                    zpool.tile([128, 512], f32, tag=f"z{n}", name=f"zb{n}")
                    for n in range(4)
                ]
                # inject host-precomputed zx (= [x,1] @ Wx_aug) via identity
                for n in BANK_ORDER:
                    ns = slice(n * 512, (n + 1) * 512)
                    nc.tensor.matmul(
                        zb[n][:], idr_sb[:], zx_sb[:, ns],
                        start=True, stop=(t == 0),
                    )
                if t > 0:
                    # k-outer first half (hT half 1 ready first), then finish
                    # banks in completion order
                    for k in (0, 1):
                        for n in BANK_ORDER:
                            ns = slice(n * 512, (n + 1) * 512)
                            nc.tensor.matmul(
                                zb[n][:],
                                hT_prev[:, k * 128:(k + 1) * 128],
                                wl_sb[:, k, ns],
                                start=False, stop=False,
                            )
                    for n in BANK_ORDER:
                        ns = slice(n * 512, (n + 1) * 512)
                        for k in (2, 3):
                            nc.tensor.matmul(
                                zb[n][:],
                                hT_prev[:, k * 128:(k + 1) * 128],
                                wl_sb[:, k, ns],
                                start=False, stop=(k == KH - 1),
                            )

                gates = sp.tile([128, G4], f32, tag="gates")
                for n in BANK_ORDER:
                    nc.scalar.activation(
                        gates[:, n * 512:(n + 1) * 512], zb[n][:], GATE_FUNC[n]
                    )

                # elementwise tail, pipelined in two H-halves
                t2 = sp.tile([128, H], f32, tag="t2")
                if t == 0:
                    c_new = t2
                else:
                    t1 = sp.tile([128, H], f32, tag="t1")
                    c_new = sp.tile([128, H], f32, tag="c")
                last = t == t_steps - 1
                if not last:
                    tanc = sp.tile([128, H], f32, tag="tanc")
                    h = sp.tile([128, H], ldtype, tag="h")
                    hT = sp.tile([128, H], ldtype, tag="hT")
                for hf in (0, 1):
                    sl = slice(hf * 256, (hf + 1) * 256)
                    gi = gates[:, hf * 256:hf * 256 + 256]
                    gf = gates[:, 512 + hf * 256:512 + hf * 256 + 256]
                    go = gates[:, 1024 + hf * 256:1024 + hf * 256 + 256]
                    gj = gates[:, 1536 + hf * 256:1536 + hf * 256 + 256]
                    nc.vector.tensor_mul(t2[:, sl], gi, gj)
                    if t > 0:
                        nc.vector.tensor_mul(t1[:, sl], c_prev[:, sl], gf)
                        nc.vector.tensor_add(c_new[:, sl], t1[:, sl], t2[:, sl])
                    if not last:
                        nc.scalar.activation(tanc[:, sl], c_new[:, sl], AF.Tanh)
                        nc.vector.tensor_mul(h[:, sl], tanc[:, sl], go)
                        trp = trpool.tile([128, 256], ldtype, tag="tr")
                        for kk in (0, 1):
                            ck = slice(hf * 256 + kk * 128, hf * 256 + (kk + 1) * 128)
                            nc.tensor.transpose(
                                trp[:, kk * 128:(kk + 1) * 128], h[:, ck], idr_sb[:]
                            )
                            # per-chunk copy so next step's k-MMs start ASAP
                            nc.vector.tensor_copy(
                                hT[:, ck], trp[:, kk * 128:(kk + 1) * 128]
                            )
                c_prev = c_new
                if not last:
                    # warm-keepers: dependency-free matmuls that fill the PE
                    # gap during the elementwise tail so HAM stays at 8/8
                    scr = scrpool.tile([128, 512], f32, tag="scr")
                    for fi in range(3):
                        nc.tensor.matmul(
                            scr[:], idr_sb[:], wl_sb[:, 0, 0:512],
                            start=True, stop=True,
                        )
                    hT_prev = hT
                else:
                    cb = sp.tile([128, H], bf16, tag="cb")
                    nc.vector.tensor_copy(cb[:], c_new[:])
                    trb = trbpool.tile([128, H], bf16, tag="trb")
                    for k in range(4):
                        ks = slice(k * 128, (k + 1) * 128)
                        nc.tensor.transpose(trb[:, ks], cb[:, ks], idb_sb[:])
                    cT_bf = sp.tile([128, H], bf16, tag="cT")
                    nc.vector.tensor_copy(cT_bf[:], trb[:])

        # gather cT across the 4 cores of this batch half
        nc.sync.dma_start(cgin.ap()[:], cT_bf[:])
        nc.gpsimd.collective_compute(
            "AllGather",
            mybir.AluOpType.bypass,
            replica_groups=[[0, 1, 2, 3], [4, 5, 6, 7]],
            ins=[cgin.ap()[:]],
            outs=[cgout.ap()[:]],
        )
        rnnT = sp.tile([128, 4 * H], bf16, tag="rnnT")
        for l in range(4):
            nc.sync.dma_start(rnnT[:, l * H:(l + 1) * H], cgout.ap()[l])

        # MLP in transposed layout (bf16 weights, fp32 psum accumulation)
        with tc.tile_pool(name="mlppsum", bufs=2, space="PSUM") as mp, \
             tc.tile_pool(name="l4psum", bufs=1, space="PSUM") as mp4:
            act_in = rnnT
            for li, (w_sb, b_sb, kc_n) in enumerate(
                [(w1_sb, b1_sb, 16), (w2_sb, b2_sb, 8), (w3_sb, b3_sb, 8)]
            ):
                aps = mp.tile([128, F], f32, tag="aps")
                for m in range(8):
                    ms = slice(m * 128, (m + 1) * 128)
                    for kc in range(kc_n):
                        nc.tensor.matmul(
                            aps[:, ms],
                            w_sb[:, kc, ms],
                            act_in[:, kc * 128:(kc + 1) * 128],
                            start=(kc == 0),
                            stop=False,
                        )
                    nc.tensor.matmul(
                        aps[:, ms], b_sb[0:1, ms], ones_sb[0:1, :],
                        start=False, stop=True,
                    )
                nxt = sp.tile([128, F], bf16, tag=f"a{li}")
                nc.scalar.activation(nxt[:], aps[:], AF.Tanh)
                act_in = nxt

            l4 = mp4.tile([3, 128], f32, tag="l4")
            for kc in range(8):
                nc.tensor.matmul(
                    l4[:], w4_sb[:, kc, :], act_in[:, kc * 128:(kc + 1) * 128],
                    start=(kc == 0), stop=False,
                )
            nc.tensor.matmul(l4[:], b4_sb[0:1, :], ones_sb[0:1, :],
                             start=False, stop=True)
            lg = sp.tile([3, 128], f32, tag="lg")
            nc.scalar.copy(lg[:], l4[:])
            nc.sync.dma_start(out_d[:], lg[:])

    return nc


def _pack_core_inputs(core, inputs, t_steps=T, ldt="f32r"):
    """Build the per-core in_map (numpy only)."""
    bf16 = ml_dtypes.bfloat16
    ldtype = np.float32 if ldt == "f32r" else bf16
    lstm = core % 4
    half = core // 4
    rows = slice(half * BC, (half + 1) * BC)

    if lstm < 2:
        x = np.asarray(inputs["premises"])[rows]
        W = np.asarray(inputs["W_fw_p"] if lstm == 0 else inputs["W_bw_p"])
        b = np.asarray(inputs["b_fw_p"] if lstm == 0 else inputs["b_bw_p"])
    else:
        x = np.asarray(inputs["hypotheses"])[rows]
        W = np.asarray(inputs["W_fw_h"] if lstm == 2 else inputs["W_bw_h"])
        b = np.asarray(inputs["b_fw_h"] if lstm == 2 else inputs["b_bw_h"])
    x = x[:, :t_steps]
    if lstm % 2 == 1:
        x = x[:, ::-1, :]

    # gate reorder [i, f, o, j]; fold forget_bias=1.0 into b
    perm = np.concatenate([
        np.arange(0, H), np.arange(2 * H, 3 * H),
        np.arange(3 * H, 4 * H), np.arange(H, 2 * H),
    ])
    Wp = W[:, perm].astype(np.float32)
    bp = b[perm].astype(np.float32).copy()
    bp[H:2 * H] += 1.0  # forget gate slice in new layout

    # host-precomputed x projection: zx[t, b, :] = x[b, t] @ Wx + b_aug
    zx = (
        np.ascontiguousarray(x.transpose(1, 0, 2)).reshape(t_steps * BC, E)
        @ Wp[:E]
        + bp
    ).reshape(t_steps, BC, 4 * H)

    # recurrent weight moving tiles (h chunks only)
    wl = np.zeros((128, KH, 4 * H), np.float32)
    for k in range(KH):
        wl[:, k, :] = Wp[E + k * 128: E + (k + 1) * 128]

    W1 = np.asarray(inputs["W1"]).astype(np.float32)
    W2 = np.asarray(inputs["W2"]).astype(np.float32)
    W3 = np.asarray(inputs["W3"]).astype(np.float32)
    W4 = np.asarray(inputs["W4"]).astype(np.float32)
    w1 = W1.reshape(16, 128, F).transpose(1, 0, 2).astype(bf16)
    w2 = W2.reshape(8, 128, F).transpose(1, 0, 2).astype(bf16)
    w3 = W3.reshape(8, 128, F).transpose(1, 0, 2).astype(bf16)
    w4 = W4.reshape(8, 128, 3).transpose(1, 0, 2).astype(bf16)

    return {
        "zx": zx.astype(ldtype),
        "wl": wl.astype(ldtype),
        "w1": np.ascontiguousarray(w1),
        "w2": np.ascontiguousarray(w2),
        "w3": np.ascontiguousarray(w3),
        "w4": np.ascontiguousarray(w4),
        "b1": np.asarray(inputs["b1"]).reshape(1, F).astype(bf16),
        "b2": np.asarray(inputs["b2"]).reshape(1, F).astype(bf16),
        "b3": np.asarray(inputs["b3"]).reshape(1, F).astype(bf16),
        "b4": np.asarray(inputs["b4"]).reshape(1, 3).astype(bf16),
        "ones": np.ones((1, 128), bf16),
        "identr": np.eye(128, dtype=ldtype),
        "identb": np.eye(128, dtype=bf16),
    }


def _install_ntff_shim():
    """This image's `antenv` lacks `axon_hooks`; provide it so
    run_bass_kernel_spmd(trace=True) can capture NTFF profiles."""
    import sys
    import types

    if "antenv.axon_hooks" in sys.modules:
        return
    mod = types.ModuleType("antenv.axon_hooks")
    state = {"hook": None}
    mod.set_axon_ntff_profile_hook = lambda h: state.__setitem__("hook", h)
    mod.get_axon_ntff_profile_hook = lambda: state["hook"]
    sys.modules["antenv.axon_hooks"] = mod
    try:
        from trn_agent_boot.trn_boot import _ntff_profile_via_ctypes

        mod.set_axon_ntff_profile_hook(
            _ntff_profile_via_ctypes("/opt/axon/libaxon_pjrt.so")
        )
    except Exception:
        pass


def _run(inputs, trace=False, t_steps=T, ldt="f32r"):
    if trace:
        _install_ntff_shim()
    from concourse.bass_utils import run_bass_kernel_spmd

    key = (t_steps, ldt)
    if key not in _cache:
        _cache[key] = _build_nc(t_steps, ldt)
    nc = _cache[key]
    in_maps = [_pack_core_inputs(c, inputs, t_steps, ldt) for c in range(N_CORES)]
    res = run_bass_kernel_spmd(
        nc, in_maps, list(range(N_CORES)), trace=trace
    )
    out = np.zeros((B, 3), np.float32)
    out[0:BC] = res.results[0]["logitsT"].T
    out[BC:2 * BC] = res.results[4]["logitsT"].T
    return out, res


def kernel(**inputs) -> np.ndarray:
    out, _ = _run(inputs, trace=False)
    return out


# revision 25
# speedup vs baseline: 1.1094x; 1.1094x over previous
"""BiRNN (Bowman SNLI) Trainium2 kernel.

Sharding: 8 cores = 4 LSTM directions x 2 batch halves (SPMD — same program,
per-core weights/inputs differ). Each core runs one LSTM (batch 128, T=128)
in "natural" layout: per step z = [x_t, h, 1] @ W_aug accumulated in PSUM with
stationary = x_t^T / h^T chunks and moving = weight columns (N=512, float32r).
Gates are column-reordered [i, f, o, j] host-side (so one sigmoid covers
i|f|o) and the forget bias is folded into b. h^T for the next step comes from
4 PE transposes. Final cell states are AllGathered within each batch-half
group of 4 cores, then every core runs the small MLP in transposed layout
(bf16) and writes logits^T [3, 128]; the host reads cores 0 and 4.
"""
import numpy as np
import ml_dtypes

# Harness-visible constants
B, T, E, H, F = 256, 128, 300, 512, 1024
BC = 128          # batch per core
N_CORES = 8
EP = 384          # padded x feature dim (300 x + 1 bias + pad)
KX = 3            # x stationary chunks (last has 45 valid rows)
KH = 4            # h stationary chunks

_cache = {}


def _apply_tile_patch():
    """walrus here allows ONE semaphore wait per instruction; Tile's tail
    drain (and occasionally other instructions) get more. Split extra waits
    onto same-engine NoOp carriers inserted immediately before."""
    import concourse.tile as tile
    import concourse.mybir as mybir
    from concourse.tile import ScopedClock

    if getattr(tile.TileContext, "_multiwait_patched", False):
        return

    def split_multiwait(nc):
        for f in nc.m.functions:
            for bb in f.blocks:
                insts = bb.instructions
                if not any(
                    i.sync_info is not None and len(i.sync_info.on_wait) > 1
                    for i in insts
                ):
                    continue
                new = []
                for inst in insts:
                    si = inst.sync_info
                    if si is not None and len(si.on_wait) > 1:
                        waits = list(si.on_wait)
                        for w in waits[:-1]:
                            carrier = mybir.InstNoOp(
                                name=nc.get_next_instruction_name(), ins=[], outs=[]
                            )
                            carrier.engine = inst.engine
                            carrier.sync_info = mybir.SyncInfo(
                                on_wait=[w], on_update=[]
                            )
                            nc.register_instruction(carrier, overwrite=True)
                            new.append(carrier)
                        si.on_wait = [waits[-1]]
                    new.append(inst)
                bb.instructions = new

    def _patched_drain_and_barrier(self, tick_clock, wait_clock):
        nc = self.nc
        drain_inst = nc.sync.drain()
        wait_clock.add_sem_waits(
            drain_inst.ins, ScopedClock({None: tick_clock.global_clock})
        )
        nc.all_engine_barrier()
        assert self.sems is not None
        popped = nc._tile_sem_poison_stack.pop()
        assert popped is self._sem_poison
        nc.clear_and_free_semaphores(list(self.sems.allocated().values()))
        nc.all_engine_barrier()
        split_multiwait(nc)

    tile.TileContext._drain_and_barrier = _patched_drain_and_barrier
    tile.TileContext._multiwait_patched = True


def _build_nc(t_steps=T, ldt="f32r"):
    _apply_tile_patch()
    from contextlib import ExitStack
    import concourse.bass as bass
    import concourse.tile as tile
    from concourse import mybir

    f32 = mybir.dt.float32
    f32r = mybir.dt.float32r
    bf16 = mybir.dt.bfloat16
    ldtype = {"f32r": f32r, "bf16": bf16}[ldt]
    AF = mybir.ActivationFunctionType

    nc = bass.Bass("TRN2", target_bir_lowering=False, debug=False,
                   num_devices=N_CORES)

    xt_d = nc.dram_tensor("xt", [t_steps, 128, KX * 128], ldtype, kind="ExternalInput").ap()
    wl_d = nc.dram_tensor("wl", [128, KX + KH, 4 * H], ldtype, kind="ExternalInput").ap()
    w1_d = nc.dram_tensor("w1", [128, 16, F], bf16, kind="ExternalInput").ap()
    w2_d = nc.dram_tensor("w2", [128, 8, F], bf16, kind="ExternalInput").ap()
    w3_d = nc.dram_tensor("w3", [128, 8, F], bf16, kind="ExternalInput").ap()
    w4_d = nc.dram_tensor("w4", [128, 8, 3], bf16, kind="ExternalInput").ap()
    b1_d = nc.dram_tensor("b1", [1, F], bf16, kind="ExternalInput").ap()
    b2_d = nc.dram_tensor("b2", [1, F], bf16, kind="ExternalInput").ap()
    b3_d = nc.dram_tensor("b3", [1, F], bf16, kind="ExternalInput").ap()
    b4_d = nc.dram_tensor("b4", [1, 3], bf16, kind="ExternalInput").ap()
    ones_d = nc.dram_tensor("ones", [1, 128], bf16, kind="ExternalInput").ap()
    idr_d = nc.dram_tensor("identr", [128, 128], ldtype, kind="ExternalInput").ap()
    idb_d = nc.dram_tensor("identb", [128, 128], bf16, kind="ExternalInput").ap()
    out_d = nc.dram_tensor("logitsT", [3, 128], f32, kind="ExternalOutput").ap()

    cgin = nc.dram_tensor("cgin", [128, H], bf16)
    cgout = nc.dram_tensor("cgout", [4, 128, H], bf16)

    G4 = 4 * H  # 2048

    with tile.TileContext(nc) as tc, ExitStack() as ctx:
        wp = ctx.enter_context(tc.tile_pool(name="weights", bufs=1))
        wl_sb = wp.tile([128, KX + KH, G4], ldtype, tag="wl")
        w1_sb = wp.tile([128, 16, F], bf16, tag="w1")
        w2_sb = wp.tile([128, 8, F], bf16, tag="w2")
        w3_sb = wp.tile([128, 8, F], bf16, tag="w3")
        w4_sb = wp.tile([128, 8, 3], bf16, tag="w4")
        b1_sb = wp.tile([1, F], bf16, tag="b1")
        b2_sb = wp.tile([1, F], bf16, tag="b2")
        b3_sb = wp.tile([1, F], bf16, tag="b3")
        b4_sb = wp.tile([1, 3], bf16, tag="b4")
        ones_sb = wp.tile([1, 128], bf16, tag="ones")
        idr_sb = wp.tile([128, 128], ldtype, tag="idr")
        idb_sb = wp.tile([128, 128], bf16, tag="idb")
        # LSTM-critical loads first (identity gates the first inject MMs;
        # wl chunk k gates step-1 h-MMs for that k)
        nc.sync.dma_start(idr_sb[:], idr_d[:])
        for k in range(KX + KH):
            nc.sync.dma_start(wl_sb[:, k, :], wl_d[:, k])
        # MLP weights aren't needed until after the T loop — emitted late
        # (below) so their DMA doesn't delay the LSTM start.
        _mlp_loads = [(w1_sb, w1_d), (w2_sb, w2_d), (w3_sb, w3_d),
                      (w4_sb, w4_d), (b1_sb, b1_d), (b2_sb, b2_d),
                      (b3_sb, b3_d), (b4_sb, b4_d), (ones_sb, ones_d),
                      (idb_sb, idb_d)]

        xp = ctx.enter_context(tc.tile_pool(name="xsteps", bufs=4))
        sp = ctx.enter_context(tc.tile_pool(name="state", bufs=2))

        c_prev = None
        hT_prev = None
        cT_bf = None

        # gate layout [i | f | o | j]; bank order chosen so the gates the
        # elementwise tail needs first (i, j) finish first
        BANK_ORDER = (0, 3, 1, 2)
        GATE_FUNC = {0: AF.Sigmoid, 1: AF.Sigmoid, 2: AF.Sigmoid, 3: AF.Tanh}

        with tc.tile_pool(name="zpsum", bufs=1, space="PSUM") as zpool, \
             tc.tile_pool(name="trpsum", bufs=2, space="PSUM") as trpool, \
             tc.tile_pool(name="trbpsum", bufs=1, space="PSUM") as trbpool, \
             tc.tile_pool(name="scrpsum", bufs=1, space="PSUM") as scrpool:
            for t in range(t_steps):
                if t == 2:
                    for sb_t, d in _mlp_loads:
                        nc.sync.dma_start(sb_t[:], d[:])
                xt_sb = xp.tile([128, KX * 128], ldtype, tag="xt")
                nc.sync.dma_start(xt_sb[:], xt_d[t])

                # one PSUM tile per gate bank so the next step's inject-MMs
                # only wait on that bank's ACT read, not all four
                zb = [
                    zpool.tile([128, 512], f32, tag=f"z{n}", name=f"zb{n}")
                    for n in range(4)
                ]
                for n in BANK_ORDER:
                    ns = slice(n * 512, (n + 1) * 512)
                    for k in range(KX):
                        nc.tensor.matmul(
                            zb[n][:],
                            xt_sb[:, k * 128:(k + 1) * 128],
                            wl_sb[:, k, ns],
                            start=(k == 0),
                            stop=(t == 0 and k == KX - 1),
                        )
                if t > 0:
                    # k-outer first half (hT half 1 ready first), then finish
                    # banks in completion order
                    for k in (0, 1):
                        for n in BANK_ORDER:
                            ns = slice(n * 512, (n + 1) * 512)
                            nc.tensor.matmul(
                                zb[n][:],
                                hT_prev[:, k * 128:(k + 1) * 128],
                                wl_sb[:, KX + k, ns],
                                start=False, stop=False,
                            )
                    for n in BANK_ORDER:
                        ns = slice(n * 512, (n + 1) * 512)
                        for k in (2, 3):
                            nc.tensor.matmul(
                                zb[n][:],
                                hT_prev[:, k * 128:(k + 1) * 128],
                                wl_sb[:, KX + k, ns],
                                start=False, stop=(k == KH - 1),
                            )

                gates = sp.tile([128, G4], f32, tag="gates")
                for n in BANK_ORDER:
                    nc.scalar.activation(
                        gates[:, n * 512:(n + 1) * 512], zb[n][:], GATE_FUNC[n]
                    )

                # elementwise tail, pipelined in two H-halves
                t2 = sp.tile([128, H], f32, tag="t2")
                if t == 0:
                    c_new = t2
                else:
                    t1 = sp.tile([128, H], f32, tag="t1")
                    c_new = sp.tile([128, H], f32, tag="c")
                last = t == t_steps - 1
                if not last:
                    tanc = sp.tile([128, H], f32, tag="tanc")
                    h = sp.tile([128, H], ldtype, tag="h")
                    hT = sp.tile([128, H], ldtype, tag="hT")
                for hf in (0, 1):
                    sl = slice(hf * 256, (hf + 1) * 256)
                    gi = gates[:, hf * 256:hf * 256 + 256]
                    gf = gates[:, 512 + hf * 256:512 + hf * 256 + 256]
                    go = gates[:, 1024 + hf * 256:1024 + hf * 256 + 256]
                    gj = gates[:, 1536 + hf * 256:1536 + hf * 256 + 256]
                    nc.vector.tensor_mul(t2[:, sl], gi, gj)
                    if t > 0:
                        nc.vector.tensor_mul(t1[:, sl], c_prev[:, sl], gf)
                        nc.vector.tensor_add(c_new[:, sl], t1[:, sl], t2[:, sl])
                    if not last:
                        nc.scalar.activation(tanc[:, sl], c_new[:, sl], AF.Tanh)
                        nc.vector.tensor_mul(h[:, sl], tanc[:, sl], go)
                        trp = trpool.tile([128, 256], ldtype, tag="tr")
                        for kk in (0, 1):
                            ck = slice(hf * 256 + kk * 128, hf * 256 + (kk + 1) * 128)
                            nc.tensor.transpose(
                                trp[:, kk * 128:(kk + 1) * 128], h[:, ck], idr_sb[:]
                            )
                            # per-chunk copy so next step's k-MMs start ASAP
                            nc.vector.tensor_copy(
                                hT[:, ck], trp[:, kk * 128:(kk + 1) * 128]
                            )
                c_prev = c_new
                if not last:
                    hT_prev = hT
                else:
                    cb = sp.tile([128, H], bf16, tag="cb")
                    nc.vector.tensor_copy(cb[:], c_new[:])
                    trb = trbpool.tile([128, H], bf16, tag="trb")
                    for k in range(4):
                        ks = slice(k * 128, (k + 1) * 128)
                        nc.tensor.transpose(trb[:, ks], cb[:, ks], idb_sb[:])
                    cT_bf = sp.tile([128, H], bf16, tag="cT")
                    nc.vector.tensor_copy(cT_bf[:], trb[:])

        # gather cT across the 4 cores of this batch half
        nc.sync.dma_start(cgin.ap()[:], cT_bf[:])
        nc.gpsimd.collective_compute(
            "AllGather",
            mybir.AluOpType.bypass,
            replica_groups=[[0, 1, 2, 3], [4, 5, 6, 7]],
            ins=[cgin.ap()[:]],
            outs=[cgout.ap()[:]],
        )
        rnnT = sp.tile([128, 4 * H], bf16, tag="rnnT")
        for l in range(4):
            nc.sync.dma_start(rnnT[:, l * H:(l + 1) * H], cgout.ap()[l])

        # MLP in transposed layout (bf16 weights, fp32 psum accumulation)
        with tc.tile_pool(name="mlppsum", bufs=2, space="PSUM") as mp, \
             tc.tile_pool(name="l4psum", bufs=1, space="PSUM") as mp4:
            act_in = rnnT
            for li, (w_sb, b_sb, kc_n) in enumerate(
                [(w1_sb, b1_sb, 16), (w2_sb, b2_sb, 8), (w3_sb, b3_sb, 8)]
            ):
                aps = mp.tile([128, F], f32, tag="aps")
                for m in range(8):
                    ms = slice(m * 128, (m + 1) * 128)
                    for kc in range(kc_n):
                        nc.tensor.matmul(
                            aps[:, ms],
                            w_sb[:, kc, ms],
                            act_in[:, kc * 128:(kc + 1) * 128],
                            start=(kc == 0),
                            stop=False,
                        )
                    nc.tensor.matmul(
                        aps[:, ms], b_sb[0:1, ms], ones_sb[0:1, :],
                        start=False, stop=True,
                    )
                nxt = sp.tile([128, F], bf16, tag=f"a{li}")
                nc.scalar.activation(nxt[:], aps[:], AF.Tanh)
                act_in = nxt

            l4 = mp4.tile([3, 128], f32, tag="l4")
            for kc in range(8):
                nc.tensor.matmul(
                    l4[:], w4_sb[:, kc, :], act_in[:, kc * 128:(kc + 1) * 128],
                    start=(kc == 0), stop=False,
                )
            nc.tensor.matmul(l4[:], b4_sb[0:1, :], ones_sb[0:1, :],
                             start=False, stop=True)
            lg = sp.tile([3, 128], f32, tag="lg")
            nc.scalar.copy(lg[:], l4[:])
            nc.sync.dma_start(out_d[:], lg[:])

    return nc


def _pack_core_inputs(core, inputs, t_steps=T, ldt="f32r"):
    """Build the per-core in_map (numpy only)."""
    bf16 = ml_dtypes.bfloat16
    ldtype = np.float32 if ldt == "f32r" else bf16
    lstm = core % 4
    half = core // 4
    rows = slice(half * BC, (half + 1) * BC)

    if lstm < 2:
        x = np.asarray(inputs["premises"])[rows]
        W = np.asarray(inputs["W_fw_p"] if lstm == 0 else inputs["W_bw_p"])
        b = np.asarray(inputs["b_fw_p"] if lstm == 0 else inputs["b_bw_p"])
    else:
        x = np.asarray(inputs["hypotheses"])[rows]
        W = np.asarray(inputs["W_fw_h"] if lstm == 2 else inputs["W_bw_h"])
        b = np.asarray(inputs["b_fw_h"] if lstm == 2 else inputs["b_bw_h"])
    x = x[:, :t_steps]
    if lstm % 2 == 1:
        x = x[:, ::-1, :]

    # gate reorder [i, f, o, j]; fold forget_bias=1.0 into b
    perm = np.concatenate([
        np.arange(0, H), np.arange(2 * H, 3 * H),
        np.arange(3 * H, 4 * H), np.arange(H, 2 * H),
    ])
    Wp = W[:, perm].astype(np.float32)
    bp = b[perm].astype(np.float32).copy()
    bp[H:2 * H] += 1.0  # forget gate slice in new layout

    xa = np.zeros((BC, t_steps, EP), np.float32)
    xa[:, :, :E] = x
    xa[:, :, E] = 1.0
    xt = np.ascontiguousarray(
        xa.reshape(BC, t_steps, KX, 128).transpose(1, 3, 2, 0)
    ).reshape(t_steps, 128, KX * 128)

    wl = np.zeros((128, KX + KH, 4 * H), np.float32)
    W_aug_x = np.zeros((EP, 4 * H), np.float32)
    W_aug_x[:E] = Wp[:E]
    W_aug_x[E] = bp
    for k in range(KX):
        wl[:, k, :] = W_aug_x[k * 128:(k + 1) * 128]
    for k in range(KH):
        wl[:, KX + k, :] = Wp[E + k * 128: E + (k + 1) * 128]

    W1 = np.asarray(inputs["W1"]).astype(np.float32)
    W2 = np.asarray(inputs["W2"]).astype(np.float32)
    W3 = np.asarray(inputs["W3"]).astype(np.float32)
    W4 = np.asarray(inputs["W4"]).astype(np.float32)
    w1 = W1.reshape(16, 128, F).transpose(1, 0, 2).astype(bf16)
    w2 = W2.reshape(8, 128, F).transpose(1, 0, 2).astype(bf16)
    w3 = W3.reshape(8, 128, F).transpose(1, 0, 2).astype(bf16)
    w4 = W4.reshape(8, 128, 3).transpose(1, 0, 2).astype(bf16)

    return {
        "xt": xt.astype(ldtype),
        "wl": wl.astype(ldtype),
        "w1": np.ascontiguousarray(w1),
        "w2": np.ascontiguousarray(w2),
        "w3": np.ascontiguousarray(w3),
        "w4": np.ascontiguousarray(w4),
        "b1": np.asarray(inputs["b1"]).reshape(1, F).astype(bf16),
        "b2": np.asarray(inputs["b2"]).reshape(1, F).astype(bf16),
        "b3": np.asarray(inputs["b3"]).reshape(1, F).astype(bf16),
        "b4": np.asarray(inputs["b4"]).reshape(1, 3).astype(bf16),
        "ones": np.ones((1, 128), bf16),
        "identr": np.eye(128, dtype=ldtype),
        "identb": np.eye(128, dtype=bf16),
    }


def _install_ntff_shim():
    """This image's `antenv` lacks `axon_hooks`; provide it so
    run_bass_kernel_spmd(trace=True) can capture NTFF profiles."""
    import sys
    import types

    if "antenv.axon_hooks" in sys.modules:
        return
    mod = types.ModuleType("antenv.axon_hooks")
    state = {"hook": None}
    mod.set_axon_ntff_profile_hook = lambda h: state.__setitem__("hook", h)
    mod.get_axon_ntff_profile_hook = lambda: state["hook"]
    sys.modules["antenv.axon_hooks"] = mod
    try:
        from trn_agent_boot.trn_boot import _ntff_profile_via_ctypes

        mod.set_axon_ntff_profile_hook(
            _ntff_profile_via_ctypes("/opt/axon/libaxon_pjrt.so")
        )
    except Exception:
        pass


def _run(inputs, trace=False, t_steps=T, ldt="f32r"):
    if trace:
        _install_ntff_shim()
    from concourse.bass_utils import run_bass_kernel_spmd

    key = (t_steps, ldt)
    if key not in _cache:
        _cache[key] = _build_nc(t_steps, ldt)
    nc = _cache[key]
    in_maps = [_pack_core_inputs(c, inputs, t_steps, ldt) for c in range(N_CORES)]
    res = run_bass_kernel_spmd(
        nc, in_maps, list(range(N_CORES)), trace=trace
    )
    out = np.zeros((B, 3), np.float32)
    out[0:BC] = res.results[0]["logitsT"].T
    out[BC:2 * BC] = res.results[4]["logitsT"].T
    return out, res


def kernel(**inputs) -> np.ndarray:
    out, _ = _run(inputs, trace=False)
    return out
